# revision 42
# baseline (speedup 1.0000x reference)
"""Trainium2 Bass kernel for causal GQA self-attention (B=2,S=2048,D=1024,H=16,HKV=4,HD=64).

Sharding: 8 cores = DP(2 over batch) x TP(4 over GQA groups).
Each core computes, for one batch element and one GQA group (4 q heads + 1 kv head),
the partial output  y_group @ Wo[:, group_cols].T  (row-sharded Wo).
Host sums the 4 TP partials per batch element.

v2 layout:
  - q4 [64, (head, S)] head-major: scores for all 4 heads = 2 matmuls of N=512 per k-tile
  - PV split into even-pair / odd-pair matmuls; odd pair's y^T lands at partitions 64:128
    (stationary operand vob has v at cols 64:128, denom-ones at col 32)
  - denominator reciprocal broadcast via GpSimd partition_broadcast (PE/ACT stay busy)
  - output projection folded into the attention block loop
"""

import os
import sys
from contextlib import ExitStack

sys.path.insert(0, "/opt/trn_rl_repo")

import numpy as np
import ml_dtypes

import concourse.bass as bass
import concourse.bacc as bacc
import concourse.tile as tile
import concourse.mybir as mybir
from concourse.bass_utils import run_bass_kernel_spmd

BF16 = mybir.dt.bfloat16
F32 = mybir.dt.float32
AF = mybir.ActivationFunctionType
ALU = mybir.AluOpType
BF16NP = ml_dtypes.bfloat16

D, H, HKV, HD, B, S = 1024, 16, 4, 64, 2, 2048
HG = 4              # q heads per core
E = HG * HD         # 256 local q-proj dim
ROPE_BASE = 10000.0
EPS = float(np.finfo(np.float32).eps)

NK = D // 128       # 8 contraction tiles for projections
SQB = 256           # sq block size in attention
NB = S // SQB       # 8 blocks
NJ = S // 128       # 16 sk tiles
HORD = [0, 2, 1, 3]  # head order along q4's head axis (even pair first)

KBCAST = os.environ.get("KBCAST", "gp")   # pe: matmul bcast | gp: gpsimd bcast
KDEBUG = int(os.environ.get("KDEBUG", "0"))


def _consts():
    i = np.arange(32, dtype=np.float64)
    inv_freq = 1.0 / (ROPE_BASE ** (2.0 * i / HD))
    pos = np.arange(S, dtype=np.float64)
    fr = pos[:, None] * inv_freq[None, :]           # [S, 32]
    cosT = np.cos(fr).T.astype(np.float32)           # [32, S]
    sinT = np.sin(fr).T.astype(np.float32)
    cos4 = np.tile(cosT, (4, 1)).astype(BF16NP)      # [128, S]
    sin4 = np.tile(sinT, (4, 1)).astype(BF16NP)

    # causal masks for diagonal sk-tiles: pattern p in {0,1}
    # valid iff c >= 128*p + r   (r: sk row 0..127, c: sq col 0..255)
    r = np.arange(128)[:, None]
    c = np.arange(SQB)[None, :]
    masks = []
    for p in range(2):
        m = (c >= 128 * p + r).astype(BF16NP)        # [128, 256]
        masks.append(np.tile(m, (1, HG)))            # [128, 1024]

    sel4x = np.zeros((128, 33), dtype=BF16NP)        # head sumsq selector (q)
    for h in range(4):
        sel4x[32 * h:32 * h + 32, h] = 1.0
    selk = np.zeros((64, 33), dtype=BF16NP)          # k sumsq -> row 32
    selk[:, 32] = 1.0
    bsel4 = np.zeros((4, 128), dtype=BF16NP)         # f[h] -> rows 32h..32h+32
    for h in range(4):
        bsel4[h, 32 * h:32 * h + 32] = 1.0
    onesk = np.zeros((33, 64), dtype=BF16NP)         # row32 ones (pe-bcast path)
    onesk[32, :] = 1.0
    ones1 = np.zeros((33, 64), dtype=np.float32)     # f32 lhsT for pe bcast
    ones1[0, :] = 1.0
    ones1[32, :] = 1.0
    id128 = np.eye(128, dtype=BF16NP)
    return cos4, sin4, masks, sel4x, selk, bsel4, onesk, ones1, id128


def _build():
    nc = bacc.Bacc("TRN2", debug=False)

    xT_d = nc.dram_tensor("xT", [D, S], BF16, kind="ExternalInput")
    wq_d = nc.dram_tensor("wq", [NK, 128, E], BF16, kind="ExternalInput")
    wkv_d = nc.dram_tensor("wkv", [NK, 128, 128], BF16, kind="ExternalInput")
    wo_d = nc.dram_tensor("wo", [2, 128, D], BF16, kind="ExternalInput")
    qlnb_d = nc.dram_tensor("qlnb", [4, 1], F32, kind="ExternalInput")
    out_d = nc.dram_tensor("out", [S, D], F32, kind="ExternalOutput")

    dbg = {}
    if KDEBUG:
        for nm, shp in [("d_q4", [64, HG, S]), ("d_k4", [64, S]),
                        ("d_qsb0", [128, S]), ("d_qsb1", [128, S]),
                        ("d_kvsb", [128, S]), ("d_veb", [128, NJ, 65]),
                        ("d_vob", [128, NJ, 128]), ("d_yn0", [128, S]),
                        ("d_yn1", [128, S]), ("d_pt", [128, 2, HG * SQB]),
                        ("d_fbcq", [128, S])]:
            dbg[nm] = nc.dram_tensor(nm, shp, BF16, kind="ExternalOutput")
        dbg["d_rb"] = nc.dram_tensor("d_rb", [128, 2, 512], F32,
                                     kind="ExternalOutput")
        dbg["d_ds"] = nc.dram_tensor("d_ds", [1, 1024], F32,
                                     kind="ExternalOutput")
        dbg["d_ytO"] = nc.dram_tensor("d_ytO", [128, 512], F32,
                                      kind="ExternalOutput")

    cos4, sin4, masks, sel4x, selk, bsel4, onesk, ones1, id128 = _consts()
    cos4_d = nc.inline_tensor(cos4, "cos4")
    sin4_d = nc.inline_tensor(sin4, "sin4")
    mask_d = [nc.inline_tensor(masks[p], f"mask{p}") for p in range(2)]
    sel4x_d = nc.inline_tensor(sel4x, "sel4x")
    selk_d = nc.inline_tensor(selk, "selk")
    bsel4_d = nc.inline_tensor(bsel4, "bsel4")
    id128_d = nc.inline_tensor(id128, "id128")

    with tile.TileContext(nc) as tc, ExitStack() as ctx:
        sp = ctx.enter_context(tc.tile_pool(name="static", bufs=1))

        def stile(shape, dt, tag):
            return sp.tile(shape, dt, name=tag, tag=tag)

        # ---- static SBUF tensors ----
        xt = [stile([128, S], BF16, f"xt{k}") for k in range(NK)]
        wq = stile([128, NK, E], BF16, "wq")
        wkv = stile([128, NK, 128], BF16, "wkv")
        wo = stile([128, 2, D], BF16, "wo")
        cos4_s = stile([128, S], BF16, "cos4")
        sin4_s = stile([128, S], BF16, "sin4")
        mask_s = [stile([128, HG * SQB], BF16, f"mask{p}") for p in range(2)]
        sel4x_s = stile([128, 33], BF16, "sel4x")
        selk_s = stile([64, 33], BF16, "selk")
        bsel4_s = stile([4, 128], BF16, "bsel4")
        id128_s = stile([128, 128], BF16, "id128")
        epsb = stile([33, 1], F32, "epsb")
        qlnb33 = stile([33, 1], F32, "qlnb33")

        qsb = [stile([128, S], BF16, f"qsb{m}") for m in range(2)]   # packed T/B
        kvsb = stile([128, S], BF16, "kvsb")                         # k(0:64) | v(64:128)
        qr = [stile([128, S], BF16, f"qr{m}") for m in range(2)]     # rotated T/B
        q4 = stile([64, HG, S], BF16, "q4")                          # head-major q
        k4 = stile([64, S], BF16, "k4")                              # rotated k
        veb = stile([128, NJ, 65], BF16, "veb")                      # [v | ones]
        vob = stile([128, NJ, 128], BF16, "vob")                     # [0|1@32|0|v]
        yn = stile([128, 2, S], BF16, "yn")                          # normalized y^T
        onesk_s = stile([33, 64], BF16, "onesk")
        fkrow = stile([16, 128], BF16, "fkrow")                      # k factors, row-major
        fkT = stile([128, NJ], F32, "fkT")                           # k factor per sk row

        # ---- load everything ----
        for k in range(NK):
            nc.sync.dma_start(xt[k][:], xT_d[128 * k:128 * (k + 1), :])
            nc.sync.dma_start(wq[:, k, :], wq_d[k])
            nc.sync.dma_start(wkv[:, k, :], wkv_d[k])
        nc.sync.dma_start(wo[:, 0, :], wo_d[0])
        nc.sync.dma_start(wo[:, 1, :], wo_d[1])
        nc.sync.dma_start(cos4_s[:], cos4_d[:])
        nc.sync.dma_start(sin4_s[:], sin4_d[:])
        for p in range(2):
            nc.sync.dma_start(mask_s[p][:], mask_d[p][:])
        nc.sync.dma_start(sel4x_s[:], sel4x_d[:])
        nc.sync.dma_start(selk_s[:], selk_d[:])
        nc.sync.dma_start(bsel4_s[:], bsel4_d[:])
        nc.sync.dma_start(id128_s[:], id128_d[:])
        onesk_d = nc.inline_tensor(onesk, "onesk")
        nc.sync.dma_start(onesk_s[:], onesk_d[:])
        nc.vector.memset(epsb[:], EPS)
        nc.vector.memset(qlnb33[:], 1.0)   # row 32 (k factor): scale 1.0
        nc.sync.dma_start(qlnb33[0:4, :], qlnb_d[:])
        nc.vector.memset(veb[:], 1.0)     # ones col at [:, j, 64]; v overwrites 0:64
        nc.vector.memset(vob[:], 0.0)
        nc.vector.memset(vob[:, :, 32:33], 1.0)

        if KBCAST == "pe":
            ones1_d = nc.inline_tensor(ones1, "ones1")
            ones1_s = stile([33, 64], F32, "ones1")
            nc.sync.dma_start(ones1_s[:], ones1_d[:])

        # ======== phase 1: projections + rms factors + rope ========
        with (
            tc.tile_pool(name="pp", bufs=6, space=bass.MemorySpace.PSUM) as pp,
            tc.tile_pool(name="misc", bufs=2, space=bass.MemorySpace.PSUM) as mp,
            tc.tile_pool(name="sq", bufs=2) as sqp,
            tc.tile_pool(name="lns", bufs=2) as lns,
            tc.tile_pool(name="rp", bufs=2) as rp,
        ):
            def emit_copies(n, pqs):
                sl = slice(512 * n, 512 * (n + 1))
                # copies PSUM -> SBUF bf16 on ACT (Copy/Square/Sqrt all
                # live in the sqrt table set, so no table churn)
                nc.scalar.copy(qsb[0][:, sl], pqs[0][:])
                nc.scalar.copy(qsb[1][:, sl], pqs[1][:])
                nc.scalar.copy(kvsb[:, sl], pqs[2][:])

            def make_rest(n):
                def rest():
                    sl = slice(512 * n, 512 * (n + 1))
                    # squares split ACT/DVE for balance
                    sq0 = sqp.tile([128, 512], BF16, name="sq0", tag="sq0")
                    sq1 = sqp.tile([128, 512], BF16, name="sq1", tag="sq1")
                    sqk = sqp.tile([64, 512], BF16, name="sqk", tag="sqk")
                    nc.scalar.square(sq0[:], qsb[0][:, sl])
                    nc.scalar.square(sq1[:], qsb[1][:, sl])
                    nc.vector.tensor_mul(sqk[:], kvsb[0:64, sl],
                                         kvsb[0:64, sl])
                    # per-head sumsq: rows 0:4 = q heads, row 32 = k
                    fpt = mp.tile([33, 512], F32, name="misc", tag="misc")
                    nc.tensor.matmul(fpt[:], sel4x_s[:], sq0[:],
                                     start=True, stop=False)
                    nc.tensor.matmul(fpt[:], sel4x_s[:], sq1[:],
                                     start=False, stop=False)
                    nc.tensor.matmul(fpt[:], selk_s[:], sqk[:],
                                     start=False, stop=True)
                    # f = (gain/8) * rsqrt(ssq/HD + eps) via Sqrt (ACT) +
                    # reciprocal (DVE) + per-head scale (row32: scale 1.0)
                    srt = lns.tile([33, 512], F32, name="srt", tag="srt")
                    nc.scalar.activation(srt[:], fpt[:], AF.Sqrt,
                                         scale=1.0 / HD, bias=epsb[:, :])
                    rcf = lns.tile([33, 512], F32, name="rcf", tag="rcf")
                    nc.vector.reciprocal_approx_fast(rcf[:], srt[:])
                    fsb = lns.tile([33, 512], BF16, name="fsb", tag="fsb")
                    nc.vector.tensor_scalar_mul(fsb[:], rcf[:], qlnb33[:, :])
                    # stage k factors for the transposed [128, NJ] table
                    nc.sync.dma_start(fkrow[4 * n:4 * n + 4, :], fsb[32:33, :])
                    # broadcast q factors to 128 rows via PE
                    pb = mp.tile([128, 512], F32, name="misc", tag="misc")
                    nc.tensor.matmul(pb[:], bsel4_s[:], fsb[0:4, :],
                                     start=True, stop=True)
                    fbcq = rp.tile([128, 512], BF16, name="fbcq", tag="fbcq")
                    nc.scalar.copy(fbcq[:], pb[:])
                    if KDEBUG:
                        nc.sync.dma_start(dbg["d_fbcq"][:, sl], fbcq[:])
                    # rope q (packed layout), f pre-multiplied
                    q0f = rp.tile([128, 512], BF16, name="q0f", tag="q0f")
                    q1f = rp.tile([128, 512], BF16, name="q1f", tag="q1f")
                    nc.vector.tensor_mul(q0f[:], qsb[0][:, sl], fbcq[:])
                    nc.vector.tensor_mul(q1f[:], qsb[1][:, sl], fbcq[:])
                    t0 = rp.tile([128, 512], BF16, name="t0", tag="t0")
                    t1 = rp.tile([128, 512], BF16, name="t1", tag="t1")
                    nc.vector.tensor_mul(t0[:], q0f[:], cos4_s[:, sl])
                    nc.vector.tensor_mul(t1[:], q1f[:], sin4_s[:, sl])
                    nc.vector.tensor_add(qr[0][:, sl], t0[:], t1[:])
                    u0 = rp.tile([128, 512], BF16, name="u0", tag="u0")
                    u1 = rp.tile([128, 512], BF16, name="u1", tag="u1")
                    nc.vector.scalar_tensor_tensor(
                        u0[:], q0f[:], -1.0, sin4_s[:, sl], ALU.mult, ALU.mult)
                    nc.vector.tensor_mul(u1[:], q1f[:], cos4_s[:, sl])
                    nc.vector.tensor_add(qr[1][:, sl], u0[:], u1[:])
                    # rope k -> k4 directly (rows 0:32 top, 32:64 bottom);
                    # the rms factor is applied later via the exp scale AP
                    ka = rp.tile([32, 512], BF16, name="ka", tag="ka")
                    kb = rp.tile([32, 512], BF16, name="kb", tag="kb")
                    nc.vector.tensor_mul(ka[:], kvsb[0:32, sl],
                                         cos4_s[0:32, sl])
                    nc.vector.tensor_mul(kb[:], kvsb[32:64, sl],
                                         sin4_s[32:64, sl])
                    nc.vector.tensor_add(k4[0:32, sl], ka[:], kb[:])
                    kc = rp.tile([32, 512], BF16, name="kc", tag="kc")
                    kd = rp.tile([32, 512], BF16, name="kd", tag="kd")
                    nc.vector.scalar_tensor_tensor(
                        kc[:], kvsb[0:32, sl], -1.0, sin4_s[0:32, sl],
                        ALU.mult, ALU.mult)
                    nc.vector.tensor_mul(kd[:], kvsb[32:64, sl],
                                         cos4_s[32:64, sl])
                    nc.vector.tensor_add(k4[32:64, sl], kc[:], kd[:])
                    # v transpose into veb / vob
                    for t in range(4):
                        st = 4 * n + t
                        ptr = mp.tile([128, 64], BF16, name="misc", tag="misc")
                        nc.tensor.transpose(
                            ptr[:], kvsb[64:128, 128 * st:128 * (st + 1)],
                            id128_s[64:128, 64:128])
                        nc.vector.tensor_copy(veb[:, st, 0:64], ptr[:])
                        nc.vector.tensor_copy(vob[:, st, 64:128], ptr[:])
                    # q reshuffle to head-major for this n-tile
                    for s4 in range(4):
                        nc.sync.dma_start(q4[0:32, s4, sl],
                                          qr[0][32 * s4:32 * s4 + 32, sl])
                        nc.sync.dma_start(q4[32:64, s4, sl],
                                          qr[1][32 * s4:32 * s4 + 32, sl])
                return rest

            rest_q = []
            for ch in range(2):           # column halves of 1024
                # --- projections (k outer for LDW reuse) ---
                pq = [[pp.tile([128, 512], F32, name="pq", tag="pq", bufs=6)
                       for _ in range(2)] for _ in range(3)]
                for k in range(NK):
                    for m in range(3):
                        lhsT = (wq[:, k, 0:128] if m == 0 else
                                wq[:, k, 128:256] if m == 1 else wkv[:, k, :])
                        for n2 in range(2):
                            c0 = 1024 * ch + 512 * n2
                            nc.tensor.matmul(
                                pq[m][n2][:], lhsT, xt[k][:, c0:c0 + 512],
                                start=(k == 0), stop=(k == NK - 1))
                # previous half's epilogue sits behind this half's matmuls
                # in the PE queue, keeping the PE warm
                for r in rest_q:
                    r()
                rest_q = []
                for n2 in range(2):
                    n = 2 * ch + n2
                    emit_copies(n, [pq[m][n2] for m in range(3)])
                    rest_q.append(make_rest(n))
            for r in rest_q:
                r()
            # transpose k factors into per-sk-row column table
            pfk = mp.tile([128, 16], BF16, name="misc", tag="misc")
            nc.tensor.transpose(pfk[:], fkrow[:], id128_s[0:16, 0:16])
            nc.vector.tensor_copy(fkT[:], pfk[:])

        if KDEBUG:
            nc.sync.dma_start(dbg["d_q4"][:], q4[:])
            nc.sync.dma_start(dbg["d_k4"][:], k4[:])
            nc.sync.dma_start(dbg["d_qsb0"][:], qsb[0][:])
            nc.sync.dma_start(dbg["d_qsb1"][:], qsb[1][:])
            nc.sync.dma_start(dbg["d_kvsb"][:], kvsb[:])
            nc.sync.dma_start(dbg["d_veb"][:], veb[:])
            nc.sync.dma_start(dbg["d_vob"][:], vob[:])

        # ======== phase 2: attention + output projection ========
        with (
            tc.tile_pool(name="ps", bufs=2, space=bass.MemorySpace.PSUM) as ps,
            tc.tile_pool(name="pyE", bufs=1, space=bass.MemorySpace.PSUM) as pyE,
            tc.tile_pool(name="pyO", bufs=1, space=bass.MemorySpace.PSUM) as pyO,
            tc.tile_pool(name="pow", bufs=2, space=bass.MemorySpace.PSUM) as pow_,
            tc.tile_pool(name="pa", bufs=6) as pa,
            tc.tile_pool(name="rd", bufs=2) as rd,
            tc.tile_pool(name="ob", bufs=3) as ob,
        ):
            def emit_scores(b, j):
                sqb = slice(SQB * b, SQB * (b + 1))
                stl = ps.tile([128, HG * SQB], F32, name="st", tag="st")
                kT = k4[:, 128 * j:128 * (j + 1)]
                nc.tensor.matmul(stl[:, 0:512], kT, q4[:, 0:2, sqb],
                                 start=True, stop=True)
                nc.tensor.matmul(stl[:, 512:1024], kT, q4[:, 2:4, sqb],
                                 start=True, stop=True)
                pt = pa.tile([128, HG * SQB], BF16, name="pt", tag="pt")
                nc.scalar.activation(pt[:], stl[:], AF.Exp,
                                     scale=fkT[:, j:j + 1])
                if j >= 2 * b:
                    nc.vector.tensor_mul(pt[:], pt[:], mask_s[j - 2 * b][:])
                if KDEBUG and b == 0:
                    nc.sync.dma_start(dbg["d_pt"][:, j, :], pt[:])
                return pt

            def make_oproj(b):
                emits = []
                for ss in range(2):
                    rows = slice(SQB * b + 128 * ss, SQB * b + 128 * (ss + 1))
                    for dh in range(2):
                        dsl = slice(512 * dh, 512 * (dh + 1))

                        def po_emit(rows=rows, dsl=dsl):
                            po = pow_.tile([128, 512], F32, name="po",
                                           tag="po")
                            nc.tensor.matmul(po[:], yn[:, 0, rows],
                                             wo[:, 0, dsl],
                                             start=True, stop=False)
                            nc.tensor.matmul(po[:], yn[:, 1, rows],
                                             wo[:, 1, dsl],
                                             start=False, stop=True)
                            ot = ob.tile([128, 512], F32, name="ot", tag="ot")
                            nc.vector.tensor_copy(ot[:], po[:])
                            nc.sync.dma_start(out_d[rows, dsl], ot[:])
                        emits.append(po_emit)
                return emits

            pending = {}
            deferred = []
            for b in range(NB):
                sq = slice(SQB * b, SQB * (b + 1))
                jmax = 2 * b + 1
                ytE = pyE.tile([65, 512], F32, name="ytE", tag="ytE")
                ytO = pyO.tile([128, 512], F32, name="ytO", tag="ytO")
                for j in range(jmax + 1):
                    pt = pending.pop((b, j), None)
                    if pt is None:
                        pt = emit_scores(b, j)
                    nc.tensor.matmul(ytE[:], veb[:, j, :], pt[:, 0:512],
                                     start=(j == 0), stop=(j == jmax))
                    nc.tensor.matmul(ytO[:], vob[:, j, :], pt[:, 512:1024],
                                     start=(j == 0), stop=(j == jmax))
                    # previous block's output projection, long off the
                    # critical path by now
                    if j >= 1 and deferred:
                        deferred.pop(0)()
                while deferred:
                    deferred.pop(0)()
                # tail: normalize into yn (stage denom rows at partition 0 --
                # custom-DVE recip misreads nonzero partition bases on HW)
                ds = rd.tile([1, 1024], F32, name="ds", tag="ds")
                nc.vector.tensor_copy(ds[:, 0:512], ytE[64:65, :])
                nc.vector.tensor_copy(ds[:, 512:1024], ytO[32:33, :])
                rden = rd.tile([1, 1024], F32, name="rden", tag="rden")
                nc.vector.reciprocal_approx_fast(rden[:], ds[:])
                # lookahead: next block's first scores keep PE/ACT busy
                # across this block's normalization tail
                if b + 1 < NB:
                    for jla in range(min(4, 2 * (b + 1) + 2)):
                        pending[(b + 1, jla)] = emit_scores(b + 1, jla)
                rb = rd.tile([128, 512], F32, name="rb", tag="rb")
                if KBCAST == "gp":
                    # dst partition base must be 0 on HW: fill all 128 rows
                    # with the odd recip first, then overwrite rows 0:64
                    nc.gpsimd.partition_broadcast(rb[:], rden[0:1, 512:1024],
                                                  channels=128)
                    nc.gpsimd.partition_broadcast(rb[0:64, :], rden[0:1, 0:512],
                                                  channels=64)
                else:
                    pbb = pow_.tile([128, 512], F32, name="po", tag="po")
                    nc.tensor.matmul(pbb[0:64, :], ones1_s[0:1, :],
                                     rden[0:1, 0:512], start=True, stop=True,
                                     skip_group_check=True)
                    nc.tensor.matmul(pbb[64:128, :], ones1_s[0:1, :],
                                     rden[0:1, 512:1024], start=True, stop=True,
                                     skip_group_check=True)
                    nc.vector.tensor_copy(rb[:], pbb[:])
                if KDEBUG and b == 0:
                    nc.sync.dma_start(dbg["d_rb"][:, 0, :], rb[:])
                    nc.sync.dma_start(dbg["d_ds"][:, 0:512], ds[:, 0:512])
                    nc.sync.dma_start(dbg["d_ds"][:, 512:1024], ds[:, 512:1024])
                    ytOc = ob.tile([128, 512], F32, name="ytOc", tag="ot")
                    nc.vector.tensor_copy(ytOc[:], ytO[:])
                    nc.sync.dma_start(dbg["d_ytO"][:], ytOc[:])
                # strided dst: head pair lands in yn[:, 0, sq] / yn[:, 1, sq]
                nc.vector.tensor_mul(yn[0:64, :, sq], ytE[0:64, :],
                                     rb[0:64, :])
                nc.vector.tensor_mul(yn[64:128, :, sq], ytO[64:128, :],
                                     rb[64:128, :])
                # output projection is deferred into the next block's j-loop
                deferred = make_oproj(b)
            for e in deferred:
                e()
            if KDEBUG:
                nc.sync.dma_start(dbg["d_yn0"][:], yn[:, 0, :])
                nc.sync.dma_start(dbg["d_yn1"][:], yn[:, 1, :])

    nc.finalize()
    return nc


_NC = None


def _get_nc():
    global _NC
    if _NC is None:
        _NC = _build()
    return _NC


def _perm():
    tops = [HORD[s] * 64 + i for s in range(HG) for i in range(32)]
    bots = [HORD[s] * 64 + 32 + i for s in range(HG) for i in range(32)]
    return tops + bots


def build_inmaps(inputs):
    x = np.asarray(inputs["x"], dtype=np.float32)
    Wq = np.asarray(inputs["Wq"], dtype=np.float32)
    Wk = np.asarray(inputs["Wk"], dtype=np.float32)
    Wv = np.asarray(inputs["Wv"], dtype=np.float32)
    Wo = np.asarray(inputs["Wo"], dtype=np.float32)
    q_gain = np.asarray(inputs["q_gain"], dtype=np.float32)

    perm = _perm()
    in_maps = []
    for c in range(8):
        dp, tp = divmod(c, 4)
        xT = np.ascontiguousarray(x[dp].T).astype(BF16NP)
        wq_sel = Wq[tp * E:(tp + 1) * E].T[:, perm]          # [D, 256] permuted
        wq_t = np.ascontiguousarray(wq_sel).astype(BF16NP).reshape(NK, 128, E)
        wk_sel = Wk[tp * HD:(tp + 1) * HD].T                  # [D, 64]
        wv_sel = Wv[tp * HD:(tp + 1) * HD].T
        wkv_t = np.concatenate([wk_sel, wv_sel], axis=1).astype(BF16NP)
        wkv_t = np.ascontiguousarray(wkv_t).reshape(NK, 128, 128)
        wo_sel = Wo[:, tp * E:(tp + 1) * E].T                 # [256, D]
        wo_t = np.ascontiguousarray(wo_sel).astype(BF16NP).reshape(2, 128, D)
        g = q_gain[tp * HG:(tp + 1) * HG].astype(np.float64)[HORD]
        qlnb = (g / 8.0).astype(np.float32).reshape(4, 1)
        in_maps.append({
            "xT": xT, "wq": wq_t, "wkv": wkv_t, "wo": wo_t, "qlnb": qlnb,
        })
    return in_maps


def kernel(x, Wq, Wk, Wv, Wo, q_gain):
    in_maps = build_inmaps({"x": x, "Wq": Wq, "Wk": Wk, "Wv": Wv, "Wo": Wo,
                            "q_gain": q_gain})
    nc = _get_nc()
    res = run_bass_kernel_spmd(nc, in_maps, core_ids=list(range(8)))
    out = np.zeros((B, S, D), dtype=np.float32)
    for c in range(8):
        out[c // 4] += res.results[c]["out"]
    return out
